# revision 1
# baseline (speedup 1.0000x reference)
"""LBLHighwayBiLm Trainium2 kernel (8-core data-parallel over batch).

Layout: activations live as [D -> 2 blocks of 128 partitions, tokens on free
dim], bf16. Highway matmuls run on PE (W^T stationary, bf16, N=1024 moving
chunks into PSUM); sigmoid (+bias) and relu (+bias) evict PSUM on ACT; the
highway combine and the 5-tap causal/anticausal convs run on DVE/GPSIMD with
fp32 tap weights as immediates. Conv boundary handling via per-row halo
columns (host-prepadded for layer 0, constant pad tiles for layer 1).
Each core handles 4 batch rows; no collectives.
"""

import numpy as np
import ml_dtypes

import concourse.bacc as bacc
import concourse.tile as tile
import concourse.mybir as mybir
from concourse.bass_utils import run_bass_kernel_spmd

BF16 = mybir.dt.bfloat16
F32 = mybir.dt.float32
AOP = mybir.AluOpType
AFT = mybir.ActivationFunctionType

N_LAYERS = 2
N_HW = 2
W = 4
D = 256
B, S = 32, 1024
NCORES = 8
BLOC = B // NCORES          # 4 batch rows per core
T = BLOC * S                # 4096 tokens per core
PB = D // 128               # 2 partition blocks for D
EB = (2 * D) // 128         # 4 partition blocks for 2D
ROW0 = S + 2 * W            # layer-0 padded row (front+back halo): 1032
ROW1 = S + W                # layer-1 padded row (one halo): 1028
CH = 1024                   # token chunk = one batch row

# --- engine assignment knobs -------------------------------------------------
# conv taps 1..4 (tap 0 is tensor_scalar on DVE): engine per tap
CONV_TAP_ENGINE = ["v", "v", "v", "v"]   # STT is DVE-only (Pool lacks the opcode)
ENG_T = "v"    # t = f - r
ENG_U = "g"    # u = g * t
ENG_X1 = "v"   # x1 = u + r
RELU_ENGINE = "a"  # "a"=ACT (reads PSUM + free bias)
MM_N = 1024    # moving free dim per matmul (bf16 allows 1024)
SCRATCH_BUFS = 9
SKIP_CONV = False
SKIP_MM = False


def _eng(nc, code):
    return {"v": nc.vector, "g": nc.gpsimd}[code]


def build_bass(params):
    """params: dict of host-precomputed constant arrays / floats."""
    nc = bacc.Bacc(target_bir_lowering=False)

    x_in = nc.dram_tensor("x", [PB, 128, BLOC * ROW0], F32, kind="ExternalInput")
    out = nc.dram_tensor(
        "out", [N_LAYERS, 2, PB, 128, T], BF16, kind="ExternalOutput"
    )

    # inline constants
    wt_dram = nc.inline_tensor(params["wt"], name="wt")        # [L,2,HW,PB,128,512] bf16
    bias_dram = nc.inline_tensor(params["bias"], name="bias")  # [128, L*2*HW*EB] f32
    pad_dram = nc.inline_tensor(params["pad1"], name="pad1")   # [128, 2*PB*W] bf16
    fw = params["fwd_w"]  # [L, W+1] python floats
    bw = params["bwd_w"]

    with tile.TileContext(nc) as tc:
        consts = tc.alloc_tile_pool(name="consts", bufs=1)
        bufs = tc.alloc_tile_pool(name="bufs", bufs=1)
        scratch = tc.alloc_tile_pool(name="scratch", bufs=SCRATCH_BUFS)
        psum = tc.alloc_tile_pool(name="psum", bufs=4, space="PSUM")

        # ---- load constants -------------------------------------------------
        wt_sb = {}
        for l in range(N_LAYERS):
            for di in range(2):
                for h in range(N_HW):
                    for kb in range(PB):
                        wtt = consts.tile(
                            [128, 2 * D], BF16, tag=f"wt{l}{di}{h}{kb}",
                            name=f"wt{l}{di}{h}{kb}",
                        )
                        nc.sync.dma_start(out=wtt, in_=wt_dram[l, di, h, kb])
                        wt_sb[(l, di, h, kb)] = wtt
        bias_sb = consts.tile([128, N_LAYERS * 2 * N_HW * EB], F32, name="bias_sb")
        nc.sync.dma_start(out=bias_sb, in_=bias_dram[:, :])
        pad_sb = consts.tile([128, 2 * PB * W], BF16, name="pad_sb")
        nc.sync.dma_start(out=pad_sb, in_=pad_dram[:, :])

        def bias_ap(l, di, h, eb):
            i = ((l * 2 + di) * N_HW + h) * EB + eb
            return bias_sb[:, i:i + 1]

        # ---- layer-0 padded input (cast-load fp32 -> bf16) ------------------
        xpad0 = []
        for blk in range(PB):
            xp = bufs.tile([128, BLOC * ROW0], BF16, tag=f"xpad0_{blk}",
                           name=f"xpad0_{blk}")
            xpad0.append(xp)
        for blk in range(PB):
            for r in range(BLOC):
                nc.gpsimd.dma_start(
                    out=xpad0[blk][:, r * ROW0:(r + 1) * ROW0],
                    in_=x_in[blk, :, r * ROW0:(r + 1) * ROW0],
                )

        # conv sources per layer: (tiles, row_len, fwd_data_off, bwd_data_off)
        # layer0 rows: [front(4) | x(1024) | back(4)], fwd taps at col j,
        # bwd taps at col 4+j.
        def conv(dst, src, row_len, base_off, taps):
            """dst[blk][:, r*S:(r+1)*S] = sum_j taps[j]*src[blk][:, r*row_len+base_off+j : +S]"""
            for blk in range(PB):
                for r in range(BLOC):
                    acc = dst[blk][:, r * CH:(r + 1) * CH]
                    def sl(j):
                        o = r * row_len + base_off + j
                        return src[blk][:, o:o + CH]
                    nc.vector.tensor_scalar_mul(acc, sl(0), float(taps[0]))
                    if SKIP_CONV:
                        continue
                    for j in range(1, W + 1):
                        eng = _eng(nc, CONV_TAP_ENGINE[j - 1])
                        eng.scalar_tensor_tensor(
                            acc, sl(j), float(taps[j]), acc, AOP.mult, AOP.add
                        )

        # ---- one highway sublayer ------------------------------------------
        def highway(l, di, h, x0, x1, x1_row_len, x1_off):
            """x1[blk] rows written from x0 [PB][128, T]; x1 may be padded
            (row_len/off) or plain (row_len=CH, off=0 with contiguous rows)."""
            for tg in range(T // MM_N):
                ps = {}
                for eb in range(EB):
                    p = psum.tile([128, MM_N], F32, tag="ps", name=f"ps{l}{di}{h}{eb}{tg}")
                    for half in range(MM_N // 512):
                        for kb in range(PB if not SKIP_MM else 1):
                            nc.tensor.matmul(
                                p[:, half * 512:(half + 1) * 512],
                                lhsT=wt_sb[(l, di, h, kb)][:, eb * 128:(eb + 1) * 128],
                                rhs=x0[kb][:, tg * MM_N + half * 512:tg * MM_N + (half + 1) * 512],
                                start=(kb == 0),
                                stop=(kb == PB - 1),
                            )
                    ps[eb] = p
                # nonlin = eblks [0, PB), gate = eblks [PB, 2*PB)
                for blk in range(PB):
                    gt = scratch.tile([128, MM_N], BF16, tag="g", name=f"g{l}{di}{h}{blk}{tg}")
                    nc.scalar.activation(
                        gt, ps[PB + blk], AFT.Sigmoid,
                        bias=bias_ap(l, di, h, PB + blk), scale=1.0,
                    )
                    rt = scratch.tile([128, MM_N], BF16, tag="r", name=f"r{l}{di}{h}{blk}{tg}")
                    if RELU_ENGINE == "a":
                        nc.scalar.activation(
                            rt, ps[blk], AFT.Relu,
                            bias=bias_ap(l, di, h, blk), scale=1.0,
                        )
                    else:
                        # (nl + bias) max 0 — one fused tensor_scalar
                        _eng(nc, RELU_ENGINE).tensor_scalar(
                            rt, ps[blk], bias_ap(l, di, h, blk), 0.0,
                            AOP.add, AOP.max,
                        )
                    tt = scratch.tile([128, MM_N], BF16, tag="t", name=f"t{l}{di}{h}{blk}{tg}")
                    x0c = x0[blk][:, tg * MM_N:(tg + 1) * MM_N]
                    _eng(nc, ENG_T).tensor_tensor(tt, x0c, rt, AOP.subtract)
                    ut = scratch.tile([128, MM_N], BF16, tag="u", name=f"u{l}{di}{h}{blk}{tg}")
                    _eng(nc, ENG_U).tensor_tensor(ut, gt, tt, AOP.mult)
                    # x1 destination chunk (MM_N == CH == one batch row)
                    o = tg * x1_row_len + x1_off
                    x1c = x1[blk][:, o:o + MM_N]
                    _eng(nc, ENG_X1).tensor_tensor(x1c, ut, rt, AOP.add)

        # ---- the network ----------------------------------------------------
        f_t = {}   # conv outputs per dir
        x1a = {}   # sublayer-A outputs per dir
        for l in range(N_LAYERS):
            # conv inputs for this layer
            if l == 0:
                src = {0: (xpad0, ROW0, 0), 1: (xpad0, ROW0, W)}
            else:
                src = {0: (xpadf, ROW1, 0), 1: (xpadb, ROW1, 0)}
            for di in range(2):
                taps = fw[l] if di == 0 else bw[l]
                ft = [
                    bufs.tile([128, T], BF16, tag=f"f{di}{blk}", name=f"f{l}{di}{blk}")
                    for blk in range(PB)
                ]
                s_tiles, rl, off = src[di]
                conv(ft, s_tiles, rl, off, taps)
                f_t[di] = ft

            # allocate next-layer padded buffers (written by sublayer B)
            if l == 0:
                xpadf = [
                    bufs.tile([128, BLOC * ROW1], BF16, tag=f"xpf{blk}", name=f"xpf{blk}")
                    for blk in range(PB)
                ]
                xpadb = [
                    bufs.tile([128, BLOC * ROW1], BF16, tag=f"xpb{blk}", name=f"xpb{blk}")
                    for blk in range(PB)
                ]
                # halos: fwd front cols [0,W), bwd back cols [S, S+W)
                for blk in range(PB):
                    for r in range(BLOC):
                        nc.vector.tensor_copy(
                            xpadf[blk][:, r * ROW1:r * ROW1 + W],
                            pad_sb[:, (0 * PB + blk) * W:(0 * PB + blk + 1) * W],
                        )
                        nc.vector.tensor_copy(
                            xpadb[blk][:, r * ROW1 + S:(r + 1) * ROW1],
                            pad_sb[:, (1 * PB + blk) * W:(1 * PB + blk + 1) * W],
                        )

            for di in range(2):
                xa = [
                    bufs.tile([128, T], BF16, tag=f"xa{di}{blk}", name=f"xa{l}{di}{blk}")
                    for blk in range(PB)
                ]
                highway(l, di, 0, f_t[di], xa, CH, 0)
                x1a[di] = xa

            for di in range(2):
                if l == 0:
                    x1 = xpadf if di == 0 else xpadb
                    rl, off = ROW1, (W if di == 0 else 0)
                else:
                    # reuse the (now dead) conv-output slots for the final out
                    x1 = [
                        bufs.tile([128, T], BF16, tag=f"f{di}{blk}", name=f"xb{l}{di}{blk}")
                        for blk in range(PB)
                    ]
                    rl, off = CH, 0
                highway(l, di, 1, x1a[di], x1, rl, off)
                # DMA the layer output (strided rows for l==0 padded bufs)
                for blk in range(PB):
                    src_ap = x1[blk].rearrange("p (r c) -> p r c", c=rl)[:, :, off:off + CH]
                    dst_ap = out[l, di, blk].rearrange("p (r c) -> p r c", c=CH)
                    nc.sync.dma_start(out=dst_ap, in_=src_ap)

        psum.release()
        scratch.release()
        bufs.release()
        consts.release()

    nc.finalize()
    return nc


def _prep_params(inputs):
    fwd_hw_W = np.asarray(inputs["fwd_hw_W"], np.float32)
    bwd_hw_W = np.asarray(inputs["bwd_hw_W"], np.float32)
    # lhsT layout: [l, dir, hw, kb, 128(k), 2D(e)] = W[e, k] transposed
    wt = np.empty((N_LAYERS, 2, N_HW, PB, 128, 2 * D), np.float32)
    for l in range(N_LAYERS):
        for di, Wsrc in ((0, fwd_hw_W), (1, bwd_hw_W)):
            for h in range(N_HW):
                wT = Wsrc[l, h].T  # [D, 2D]
                wt[l, di, h] = wT.reshape(PB, 128, 2 * D)
    wt = wt.astype(ml_dtypes.bfloat16)

    fwd_hw_b = np.asarray(inputs["fwd_hw_b"], np.float32)
    bwd_hw_b = np.asarray(inputs["bwd_hw_b"], np.float32)
    bias = np.empty((128, N_LAYERS * 2 * N_HW * EB), np.float32)
    for l in range(N_LAYERS):
        for di, bsrc in ((0, fwd_hw_b), (1, bwd_hw_b)):
            for h in range(N_HW):
                for eb in range(EB):
                    i = ((l * 2 + di) * N_HW + h) * EB + eb
                    bias[:, i] = bsrc[l, h, eb * 128:(eb + 1) * 128]

    # layer-1 halos: fwd front = fwd_pad[1].T, bwd back = bwd_pad[1].T
    fwd_pad = np.asarray(inputs["fwd_pad"], np.float32)
    bwd_pad = np.asarray(inputs["bwd_pad"], np.float32)
    pad1 = np.empty((128, 2 * PB * W), np.float32)
    for di, psrc in ((0, fwd_pad), (1, bwd_pad)):
        pT = psrc[1].T.reshape(PB, 128, W)  # [D, W] -> blocks
        for blk in range(PB):
            pad1[:, (di * PB + blk) * W:(di * PB + blk + 1) * W] = pT[blk]
    pad1 = pad1.astype(ml_dtypes.bfloat16)

    return {
        "wt": np.ascontiguousarray(wt),
        "bias": np.ascontiguousarray(bias),
        "pad1": np.ascontiguousarray(pad1),
        "fwd_w": [[float(v) for v in row] for row in np.asarray(inputs["fwd_w"], np.float32)],
        "bwd_w": [[float(v) for v in row] for row in np.asarray(inputs["bwd_w"], np.float32)],
    }


def _prep_core_input(x_core, fwd_pad, bwd_pad):
    """x_core: [BLOC, S, D] f32 -> [PB, 128, BLOC*ROW0] f32 with halos."""
    xt = np.ascontiguousarray(x_core.transpose(2, 0, 1))  # [D, BLOC, S]
    blocks = xt.reshape(PB, 128, BLOC, S)
    padded = np.empty((PB, 128, BLOC, ROW0), np.float32)
    padded[:, :, :, W:W + S] = blocks
    fr = fwd_pad[0].T.reshape(PB, 128, W)   # front halo (layer 0)
    bk = bwd_pad[0].T.reshape(PB, 128, W)
    padded[:, :, :, :W] = fr[:, :, None, :]
    padded[:, :, :, W + S:] = bk[:, :, None, :]
    return np.ascontiguousarray(padded.reshape(PB, 128, BLOC * ROW0))


_NC_CACHE = {}


def kernel(**inputs):
    params = _prep_params(inputs)
    import hashlib
    h = hashlib.sha256()
    for k in ("wt", "bias", "pad1"):
        h.update(params[k].tobytes())
    h.update(repr(params["fwd_w"]).encode())
    h.update(repr(params["bwd_w"]).encode())
    key = h.hexdigest()
    if key not in _NC_CACHE:
        _NC_CACHE[key] = build_bass(params)
    nc = _NC_CACHE[key]

    x = np.asarray(inputs["inputs"], np.float32)
    fwd_pad = np.asarray(inputs["fwd_pad"], np.float32)
    bwd_pad = np.asarray(inputs["bwd_pad"], np.float32)
    in_maps = [
        {"x": _prep_core_input(x[c * BLOC:(c + 1) * BLOC], fwd_pad, bwd_pad)}
        for c in range(NCORES)
    ]
    res = run_bass_kernel_spmd(nc, in_maps, core_ids=list(range(NCORES)))

    y = np.empty((N_LAYERS, B, S, 2 * D), np.float32)
    for c in range(NCORES):
        o = np.asarray(res.results[c]["out"]).astype(np.float32)
        # [L, dir, blk, p, T] -> [L, r, s, dir*256+blk*128+p]
        o = o.reshape(N_LAYERS, 2, PB, 128, BLOC, S)
        o = o.transpose(0, 4, 5, 1, 2, 3).reshape(N_LAYERS, BLOC, S, 2 * D)
        y[:, c * BLOC:(c + 1) * BLOC] = o
    return y



# revision 7
# speedup vs baseline: 1.1919x; 1.1919x over previous
"""LBLHighwayBiLm Trainium2 kernel (8-core data-parallel over batch).

v2 layout: activations [D -> 2 blocks of 128 partitions, tokens on free dim],
bf16, with a uniform padded row stride of 1032 (4 halo/slack + 1024 data + 4)
for every conv source/dest so the 5-tap convs run as whole-block sliding ops.

Engine split (per cost model):
- Highway matmuls on PE (bf16, 512-col chunks into [128,1024] PSUM tiles).
- Conv: per (layer, dir, d-block) either PE (5 accumulating diagonal-lhsT
  matmuls per 512-chunk, PSUM evict via knob engine) or DVE (tensor_scalar
  4x-mode scaled copies + tensor_tensor 2x adds sliding over the whole
  padded block).
- Sigmoid eviction on ACT; relu eviction engine per-op knob (ACT/DVE/Pool).
- Highway combine t/x1 on DVE, u engine per-op knob (DVE/Pool).
Each core handles 4 batch rows; no collectives.
"""

import numpy as np
import ml_dtypes

import concourse.bacc as bacc
import concourse.tile as tile
import concourse.mybir as mybir
from concourse.bass_utils import run_bass_kernel_spmd

BF16 = mybir.dt.bfloat16
F32 = mybir.dt.float32
AOP = mybir.AluOpType
AFT = mybir.ActivationFunctionType

N_LAYERS = 2
N_HW = 2
W = 4
D = 256
B, S = 32, 1024
NCORES = 8
BLOC = B // NCORES          # 4 batch rows per core
T = BLOC * S                # 4096 tokens per core
PB = D // 128               # 2 partition blocks for D
EB = (2 * D) // 128         # 4 partition blocks for 2D
ROW = S + 2 * W             # uniform padded row stride: 1032
CH = 1024                   # token chunk = one batch row
WD = BLOC * ROW - 2 * W     # whole-block sliding op width: 4120

# --- engine assignment knobs -------------------------------------------------
# conv engine per (l, di, blk): "t" = PE diag-matmul path, "v" = DVE path.
# DVE conv lives on the fwd stream; highway emission runs bwd first so PE
# streams bwd matmuls while DVE finishes the fwd convs.
CONV_ENGINE = {
    (0, 0, 0): "v", (0, 0, 1): "v", (0, 1, 0): "t", (0, 1, 1): "t",
    (1, 0, 0): "v", (1, 0, 1): "v", (1, 1, 0): "t", (1, 1, 1): "t",
}
HW_DI_ORDER = (1, 0)   # bwd first
RELU_CYCLE = "aaaaad"  # relu PSUM-evict engine cycle: a=ACT, d=DVE (no Pool: PSUM)
CEVICT_CYCLE = "a"     # conv PSUM-evict engine cycle (PE conv path)
U_CYCLE = "g"          # u = g*t engine cycle: v=DVE, g=Pool
ENG_T = "v"            # t = x0 - r
ENG_X1 = "v"           # x1 = u + r
PSUM_BUFS = 4
SCRATCH_BUFS = 4


def _eng(nc, code):
    return {"v": nc.vector, "d": nc.vector, "g": nc.gpsimd, "p": nc.gpsimd,
            "a": nc.scalar}[code]


class _Cycle:
    def __init__(self, pattern):
        self.pattern = pattern
        self.i = 0

    def next(self):
        c = self.pattern[self.i % len(self.pattern)]
        self.i += 1
        return c


def build_bass(params):
    """params: dict of host-precomputed constant arrays / floats."""
    nc = bacc.Bacc(target_bir_lowering=False)

    x_in = nc.dram_tensor("x", [PB, 128, BLOC * ROW], BF16, kind="ExternalInput")
    out = nc.dram_tensor(
        "out", [N_LAYERS, 2, PB, 128, T], BF16, kind="ExternalOutput"
    )

    # inline constants
    wt_dram = nc.inline_tensor(params["wt"], name="wt")        # [L,2,HW,PB,128,512] bf16
    bias_dram = nc.inline_tensor(params["bias"], name="bias")  # [128, L*2*HW*EB] f32
    pad_dram = nc.inline_tensor(params["pad1"], name="pad1")   # [128, 2*PB*W] bf16
    dg_dram = nc.inline_tensor(params["dg"], name="dg")        # [L,2,W+1,128,128] bf16
    fw = params["fwd_w"]  # [L, W+1] python floats
    bw = params["bwd_w"]

    relu_cycle = _Cycle(RELU_CYCLE)
    cevict_cycle = _Cycle(CEVICT_CYCLE)
    u_cycle = _Cycle(U_CYCLE)

    with tile.TileContext(nc) as tc:
        consts = tc.alloc_tile_pool(name="consts", bufs=1)
        bufs = tc.alloc_tile_pool(name="bufs", bufs=1)
        scratch = tc.alloc_tile_pool(name="scratch", bufs=SCRATCH_BUFS)
        ctmp_pool = tc.alloc_tile_pool(name="ctmp", bufs=2)
        psum = tc.alloc_tile_pool(name="psum", bufs=PSUM_BUFS, space="PSUM")

        # ---- load constants -------------------------------------------------
        wt_sb = {}
        for l in range(N_LAYERS):
            for di in range(2):
                for h in range(N_HW):
                    for kb in range(PB):
                        wtt = consts.tile(
                            [128, 2 * D], BF16, tag=f"wt{l}{di}{h}{kb}",
                            name=f"wt{l}{di}{h}{kb}",
                        )
                        nc.sync.dma_start(out=wtt, in_=wt_dram[l, di, h, kb])
                        wt_sb[(l, di, h, kb)] = wtt
        dg_sb = {}
        for l in range(N_LAYERS):
            for di in range(2):
                if not any(CONV_ENGINE[(l, di, blk)] == "t" for blk in range(PB)):
                    continue
                for j in range(W + 1):
                    dgt = consts.tile([128, 128], BF16, tag=f"dg{l}{di}{j}",
                                      name=f"dg{l}{di}{j}")
                    nc.sync.dma_start(out=dgt, in_=dg_dram[l, di, j])
                    dg_sb[(l, di, j)] = dgt
        bias_sb = consts.tile([128, N_LAYERS * 2 * N_HW * EB], F32, name="bias_sb")
        nc.sync.dma_start(out=bias_sb, in_=bias_dram[:, :])
        pad_sb = consts.tile([128, 2 * PB * W], BF16, name="pad_sb")
        nc.sync.dma_start(out=pad_sb, in_=pad_dram[:, :])

        def bias_ap(l, di, h, eb):
            i = ((l * 2 + di) * N_HW + h) * EB + eb
            return bias_sb[:, i:i + 1]

        # ---- layer-0 padded input (host-prepadded bf16) ---------------------
        xpad0 = []
        for blk in range(PB):
            xp = bufs.tile([128, BLOC * ROW], BF16, tag=f"xpad0_{blk}",
                           name=f"xpad0_{blk}")
            nc.gpsimd.dma_start(out=xp, in_=x_in[blk])
            xpad0.append(xp)

        # ---- conv: one (l, di, blk) group -----------------------------------
        # src layout: uniform ROW-stride rows; fwd taps read base 0, bwd base W.
        def conv_group(l, di, blk, src_t, dst_t, taps, base):
            eng = CONV_ENGINE[(l, di, blk)]
            if eng == "v":
                # whole-block sliding ops (seam cols hold garbage, never read)
                acc = dst_t[:, 0:WD]
                nc.vector.tensor_scalar_mul(
                    acc, src_t[:, base:base + WD], float(taps[0]))
                for j in range(1, W + 1):
                    tmp = ctmp_pool.tile([128, WD], BF16, tag="ctmp",
                                         name=f"ct{l}{di}{blk}{j}")
                    nc.vector.tensor_scalar_mul(
                        tmp, src_t[:, base + j:base + j + WD], float(taps[j]))
                    nc.vector.tensor_tensor(acc, acc, tmp, AOP.add)
            else:
                # PE: per row, 5 accumulating diag matmuls per 512-chunk
                for r in range(BLOC):
                    ps = psum.tile([128, CH], F32, tag="ps",
                                   name=f"cps{l}{di}{blk}{r}")
                    for c0 in (0, 512):
                        o = r * ROW + base + c0
                        for j in range(W + 1):
                            nc.tensor.matmul(
                                ps[:, c0:c0 + 512],
                                lhsT=dg_sb[(l, di, j)],
                                rhs=src_t[:, o + j:o + j + 512],
                                start=(j == 0),
                                stop=(j == W),
                            )
                    ev = cevict_cycle.next()
                    dst_ap = dst_t[:, r * ROW:r * ROW + CH]
                    if ev == "a":
                        nc.scalar.activation(dst_ap, ps, AFT.Copy)
                    else:
                        _eng(nc, ev).tensor_copy(dst_ap, ps)

        # ---- one highway sublayer ------------------------------------------
        # x0_ap(blk, c0, n): read AP for matmul rhs / combine input
        # x1_ap(blk, tg): write AP for the combine output chunk [128, CH]
        def highway(l, di, h, x0_ap, x1_ap):
            for tg in range(T // CH):
                ps = {}
                for eb in range(EB):
                    p = psum.tile([128, CH], F32, tag="ps",
                                  name=f"ps{l}{di}{h}{eb}{tg}")
                    for kb in range(PB):
                        for half in range(CH // 512):
                            nc.tensor.matmul(
                                p[:, half * 512:(half + 1) * 512],
                                lhsT=wt_sb[(l, di, h, kb)][:, eb * 128:(eb + 1) * 128],
                                rhs=x0_ap(kb, tg * CH + half * 512, 512),
                                start=(kb == 0),
                                stop=(kb == PB - 1),
                            )
                    ps[eb] = p
                # nonlin = eblks [0, PB), gate = eblks [PB, 2*PB)
                for blk in range(PB):
                    gt = scratch.tile([128, CH], BF16, tag="g", name=f"g{l}{di}{h}{blk}{tg}")
                    nc.scalar.activation(
                        gt, ps[PB + blk], AFT.Sigmoid,
                        bias=bias_ap(l, di, h, PB + blk), scale=1.0,
                    )
                    rt = scratch.tile([128, CH], BF16, tag="r", name=f"r{l}{di}{h}{blk}{tg}")
                    re = relu_cycle.next()
                    if re == "a":
                        nc.scalar.activation(
                            rt, ps[blk], AFT.Relu,
                            bias=bias_ap(l, di, h, blk), scale=1.0,
                        )
                    else:
                        _eng(nc, re).tensor_scalar(
                            rt, ps[blk], bias_ap(l, di, h, blk), 0.0,
                            AOP.add, AOP.max,
                        )
                    tt = scratch.tile([128, CH], BF16, tag="t", name=f"t{l}{di}{h}{blk}{tg}")
                    x0c = x0_ap(blk, tg * CH, CH)
                    _eng(nc, ENG_T).tensor_tensor(tt, x0c, rt, AOP.subtract)
                    ut = scratch.tile([128, CH], BF16, tag="u", name=f"u{l}{di}{h}{blk}{tg}")
                    _eng(nc, u_cycle.next()).tensor_tensor(ut, gt, tt, AOP.mult)
                    _eng(nc, ENG_X1).tensor_tensor(x1_ap(blk, tg), ut, rt, AOP.add)

        # ---- the network ----------------------------------------------------
        # padded access: row r data at [r*ROW + off, +CH)
        def padded_x0(tiles, off):
            def f(blk, c, n):
                r, c0 = divmod(c, CH)
                return tiles[blk][:, r * ROW + off + c0:r * ROW + off + c0 + n]
            return f

        def packed_x0(tiles):
            return lambda blk, c, n: tiles[blk][:, c:c + n]

        def padded_x1(tiles, off):
            return lambda blk, tg: tiles[blk][:, tg * ROW + off:tg * ROW + off + CH]

        def packed_x1(tiles):
            return lambda blk, tg: tiles[blk][:, tg * CH:(tg + 1) * CH]

        for l in range(N_LAYERS):
            # conv sources: (tiles, fwd base, bwd base); data sits at col
            # base+W within each ROW-stride row for every buffer.
            if l == 0:
                src_f, src_b = xpad0, xpad0
            else:
                src_f, src_b = xpadf, xpadb

            # conv outputs (padded layout, data at row base 0)
            f_t = {}
            for di in range(2):
                f_t[di] = [
                    bufs.tile([128, BLOC * ROW], BF16, tag=f"f{di}{blk}",
                              name=f"f{l}{di}{blk}")
                    for blk in range(PB)
                ]
            # DVE conv groups first (engine-parallel with PE groups)
            order = sorted(
                [(l, di, blk) for di in range(2) for blk in range(PB)],
                key=lambda k: 0 if CONV_ENGINE[k] == "v" else 1,
            )
            for (_, di, blk) in order:
                taps = fw[l] if di == 0 else bw[l]
                src = src_f if di == 0 else src_b
                conv_group(l, di, blk, src[blk], f_t[di][blk], taps,
                           0 if di == 0 else W)

            # next-layer padded buffers + halo constants (layer 0 only)
            if l == 0:
                xpadf = [
                    bufs.tile([128, BLOC * ROW], BF16, tag=f"xpf{blk}", name=f"xpf{blk}")
                    for blk in range(PB)
                ]
                xpadb = [
                    bufs.tile([128, BLOC * ROW], BF16, tag=f"xpb{blk}", name=f"xpb{blk}")
                    for blk in range(PB)
                ]
                # halos: fwd front cols [0,W), bwd back cols [W+S, ROW)
                for blk in range(PB):
                    for r in range(BLOC):
                        nc.vector.tensor_copy(
                            xpadf[blk][:, r * ROW:r * ROW + W],
                            pad_sb[:, (0 * PB + blk) * W:(0 * PB + blk + 1) * W],
                        )
                        nc.vector.tensor_copy(
                            xpadb[blk][:, r * ROW + W + S:(r + 1) * ROW],
                            pad_sb[:, (1 * PB + blk) * W:(1 * PB + blk + 1) * W],
                        )

            # sublayer A: f -> xa (packed)
            xa = {}
            for di in HW_DI_ORDER:
                xa[di] = [
                    bufs.tile([128, T], BF16, tag=f"xa{di}{blk}", name=f"xa{l}{di}{blk}")
                    for blk in range(PB)
                ]
                highway(l, di, 0, padded_x0(f_t[di], 0), packed_x1(xa[di]))

            # sublayer B: xa -> padded bufs (next-layer conv src for l=0;
            # l=1 reuses the xpadf/xpadb buffers, then dead, as plain output)
            for di in HW_DI_ORDER:
                if l == 0:
                    x1t = xpadf if di == 0 else xpadb
                else:
                    tg_ = "xpf" if di == 0 else "xpb"
                    x1t = [
                        bufs.tile([128, BLOC * ROW], BF16, tag=f"{tg_}{blk}",
                                  name=f"xb{l}{di}{blk}")
                        for blk in range(PB)
                    ]
                highway(l, di, 1, packed_x0(xa[di]), padded_x1(x1t, W))
                for blk in range(PB):
                    src_ap = x1t[blk].rearrange(
                        "p (r c) -> p r c", c=ROW)[:, :, W:W + CH]
                    dst_ap = out[l, di, blk].rearrange("p (r c) -> p r c", c=CH)
                    nc.sync.dma_start(out=dst_ap, in_=src_ap)

        psum.release()
        ctmp_pool.release()
        scratch.release()
        bufs.release()
        consts.release()

    nc.finalize()
    return nc


def _prep_params(inputs):
    fwd_hw_W = np.asarray(inputs["fwd_hw_W"], np.float32)
    bwd_hw_W = np.asarray(inputs["bwd_hw_W"], np.float32)
    # lhsT layout: [l, dir, hw, kb, 128(k), 2D(e)] = W[e, k] transposed
    wt = np.empty((N_LAYERS, 2, N_HW, PB, 128, 2 * D), np.float32)
    for l in range(N_LAYERS):
        for di, Wsrc in ((0, fwd_hw_W), (1, bwd_hw_W)):
            for h in range(N_HW):
                wT = Wsrc[l, h].T  # [D, 2D]
                wt[l, di, h] = wT.reshape(PB, 128, 2 * D)
    wt = wt.astype(ml_dtypes.bfloat16)

    fwd_hw_b = np.asarray(inputs["fwd_hw_b"], np.float32)
    bwd_hw_b = np.asarray(inputs["bwd_hw_b"], np.float32)
    bias = np.empty((128, N_LAYERS * 2 * N_HW * EB), np.float32)
    for l in range(N_LAYERS):
        for di, bsrc in ((0, fwd_hw_b), (1, bwd_hw_b)):
            for h in range(N_HW):
                for eb in range(EB):
                    i = ((l * 2 + di) * N_HW + h) * EB + eb
                    bias[:, i] = bsrc[l, h, eb * 128:(eb + 1) * 128]

    # layer-1 halos: fwd front = fwd_pad[1].T, bwd back = bwd_pad[1].T
    fwd_pad = np.asarray(inputs["fwd_pad"], np.float32)
    bwd_pad = np.asarray(inputs["bwd_pad"], np.float32)
    pad1 = np.empty((128, 2 * PB * W), np.float32)
    for di, psrc in ((0, fwd_pad), (1, bwd_pad)):
        pT = psrc[1].T.reshape(PB, 128, W)  # [D, W] -> blocks
        for blk in range(PB):
            pad1[:, (di * PB + blk) * W:(di * PB + blk + 1) * W] = pT[blk]
    pad1 = pad1.astype(ml_dtypes.bfloat16)

    fwd_w = np.asarray(inputs["fwd_w"], np.float32)
    bwd_w = np.asarray(inputs["bwd_w"], np.float32)
    # diagonal tap matrices for the PE conv path
    dg = np.zeros((N_LAYERS, 2, W + 1, 128, 128), np.float32)
    for l in range(N_LAYERS):
        for di, wsrc in ((0, fwd_w), (1, bwd_w)):
            for j in range(W + 1):
                np.fill_diagonal(dg[l, di, j], wsrc[l, j])
    dg = dg.astype(ml_dtypes.bfloat16)

    return {
        "wt": np.ascontiguousarray(wt),
        "bias": np.ascontiguousarray(bias),
        "pad1": np.ascontiguousarray(pad1),
        "dg": np.ascontiguousarray(dg),
        "fwd_w": [[float(v) for v in row] for row in fwd_w],
        "bwd_w": [[float(v) for v in row] for row in bwd_w],
    }


def _prep_core_input(x_core, fwd_pad, bwd_pad):
    """x_core: [BLOC, S, D] f32 -> [PB, 128, BLOC*ROW] bf16 with halos."""
    xt = np.ascontiguousarray(x_core.transpose(2, 0, 1))  # [D, BLOC, S]
    blocks = xt.reshape(PB, 128, BLOC, S)
    padded = np.empty((PB, 128, BLOC, ROW), np.float32)
    padded[:, :, :, W:W + S] = blocks
    fr = fwd_pad[0].T.reshape(PB, 128, W)   # front halo (layer 0)
    bk = bwd_pad[0].T.reshape(PB, 128, W)
    padded[:, :, :, :W] = fr[:, :, None, :]
    padded[:, :, :, W + S:] = bk[:, :, None, :]
    return np.ascontiguousarray(
        padded.reshape(PB, 128, BLOC * ROW).astype(ml_dtypes.bfloat16))


_NC_CACHE = {}


def kernel(**inputs):
    params = _prep_params(inputs)
    import hashlib
    h = hashlib.sha256()
    for k in ("wt", "bias", "pad1", "dg"):
        h.update(params[k].tobytes())
    h.update(repr(params["fwd_w"]).encode())
    h.update(repr(params["bwd_w"]).encode())
    key = h.hexdigest()
    if key not in _NC_CACHE:
        _NC_CACHE[key] = build_bass(params)
    nc = _NC_CACHE[key]

    x = np.asarray(inputs["inputs"], np.float32)
    fwd_pad = np.asarray(inputs["fwd_pad"], np.float32)
    bwd_pad = np.asarray(inputs["bwd_pad"], np.float32)
    in_maps = [
        {"x": _prep_core_input(x[c * BLOC:(c + 1) * BLOC], fwd_pad, bwd_pad)}
        for c in range(NCORES)
    ]
    res = run_bass_kernel_spmd(nc, in_maps, core_ids=list(range(NCORES)))

    y = np.empty((N_LAYERS, B, S, 2 * D), np.float32)
    for c in range(NCORES):
        o = np.asarray(res.results[c]["out"]).astype(np.float32)
        # [L, dir, blk, p, T] -> [L, r, s, dir*256+blk*128+p]
        o = o.reshape(N_LAYERS, 2, PB, 128, BLOC, S)
        o = o.transpose(0, 4, 5, 1, 2, 3).reshape(N_LAYERS, BLOC, S, 2 * D)
        y[:, c * BLOC:(c + 1) * BLOC] = o
    return y
